# revision 34
# baseline (speedup 1.0000x reference)
"""Trainium2 Bass kernel for MultiHeadGeneralizedPooling.

Reference computation (per batch b):
  Hi   = einsum('sd,ihd->ish', X, P) + bP             (nh, S, HD)
  A    = W2 @ relu(W1 @ Hi + b1) + b2                 (nh, S, HD)
  A    = softmax(A + log(mask), axis=S)
  v    = sum_s Hi * A                                 (nh, HD)
  out  = concat_heads(v)                              (NH*HD,)

v7 strategy (evolved from the v6 bf16 baseline, kernel_v6_baseline.py;
337-375us -> ~280us):
  - KEY ALGEBRA: with em = exp(score), v_num = sum_s em*hi
      = sum_real hi  +  sum_s (em-1)*hi.
    The first term is computed EXACTLY on the host (fp32 P @ sum_s X).
    The second term carries an (em-1) ~ O(0.03) weight, so fp8 noise in
    hi contributes only ~4e-5 to v. This makes the ENTIRE on-chip
    pipeline fp8-tolerant: X, P, hi, W1, W2 all fp8.
  - fp8 DoubleRow projection: K=768 contraction packed as 3 matmuls of
    K=256 (2 k-tiles/partition) per output tile -> 18 matmuls/batch at
    0.5 cycles/moving-elem, HALF the bf16 PE time. (DoublePixel was
    measured to be a silent no-op on TRN2; fp8 without a perf mode gains
    nothing.)
  - X shipped as fp8 (half the HBM traffic of v6), one DMA per batch on
    the sync queue, prefetched one iteration ahead.
  - hi stored fp8-only as 4*hi (P host-scaled x4): paired ACT evac from
    PSUM, fp8 repartition pieces on sync (half the bytes of v6).
  - scores: W1 fp8x64 (64*b1' as 97th contraction row against the
    constant-4.0 hi row), relu evac scale 1/16 -> u = fp8(16u), W2
    fp8x64 DoubleRow, exp scale 1/1024 on ACT with f32 em + den accum.
  - weighted sum: one DVE STT per head: (em - 1) * hi8, free-dim
    accumulated -> vnum4.  v = (vnum4 + 4*mean_host)/(4*(den - corr))*4
    folded as vout = (vq4 * 0.25) + bP in a single STT.
  - 3-deep software pipeline: iteration it runs proj(b=it) | W1(b-1) |
    W2+softmax+output(b-2), fully interleaved on PE (_PE_ORDER) so every
    PSUM WAR gap is covered by ready work from another stream. Engine
    split: ACT = hi evacs(3 pairs) + exp(8) + 5 relu pairs; DVE = 7 relu
    pairs + STT(8) + reciprocal + final STT; Pool = tail elementwise;
    sync = X + repartition(15) + out. ACT/DVE are the ~15us/batch
    co-bottleneck (Pool cannot read PSUM on TRN2).
  - softmax without max-subtraction (scores ~N(0,0.03)); padded-column
    denominator overcount subtracted via host-computed corr (replicates
    the chip's exact fp8 arithmetic on a padded column).
"""

import numpy as np
import ml_dtypes

B, S, D = 128, 512, 768
NH, HD = 8, 96
HID = 4 * HD  # 384
NCORES = 8
BPC = B // NCORES  # batches per core
DC = D // 128      # 6 d-chunks
FC = HID // 128    # 3 f-chunks
HT = D // 128      # 6 concat feature tiles
NCH = NH * FC      # 24 u-chunks per batch

# fp8 DoublePixel perf mode (2 moving pixels/cycle) on the K<=128 matmuls
# (W1, W2 third chunk). CoreSim doesn't model DP, so test.py sim sets this
# False before build_module; hardware correctness is gated by rel-err.
USE_DP = False  # measured: no effect on TRN2 (silently ignored), keep off

_CACHE = {}


def _lattice_split(base, length):
    segs = []
    while length > 0:
        for sz in (128, 96, 64, 32):
            if length >= sz and (base == 0 if sz == 96 else base % sz == 0):
                segs.append((base, sz))
                base += sz
                length -= sz
                break
        else:
            raise ValueError((base, length))
    return segs


# head i occupies concatenated-feature rows [96i, 96i+96): pieces of the six
# 128-row tiles: (tile, base_partition, length, head_row_offset)
_PIECES = []
for _i in range(NH):
    lo, hi = _i * HD, (_i + 1) * HD
    ps = []
    t0, t1 = lo // 128, (hi - 1) // 128
    for _t in range(t0, t1 + 1):
        s = max(lo, _t * 128)
        e = min(hi, (_t + 1) * 128)
        for _b, _sz in _lattice_split(s - _t * 128, e - s):
            ps.append((_t, _b, _sz, _t * 128 + _b - lo))
    _PIECES.append(ps)

# pieces grouped by the tile-pair whose evac produces them
_PIECES_BY_TP = [[], [], []]
for _i in range(NH):
    for (_t, _b, _sz, _off) in _PIECES[_i]:
        _PIECES_BY_TP[_t // 2].append((_i, _t, _b, _sz, _off))

# PE issue order per iteration of the 3-deep pipeline: W2 heads (batch o),
# W1 chunk-pairs (batch m) and proj tile-pairs (batch b) interleaved so every
# PSUM WAR gap (psum_hi evac, psum_u relu, psum_a exp) is covered by ready
# work from another stream.
_PE_ORDER = [
    ("h", 0), ("pair", 0), ("w1", 0), ("h", 1), ("w1", 1), ("h", 2),
    ("w1", 2), ("pair", 1), ("w1", 3), ("h", 3), ("w1", 4), ("h", 4),
    ("w1", 5), ("pair", 2), ("w1", 6), ("h", 5), ("w1", 7), ("h", 6),
    ("w1", 8), ("w1", 9), ("h", 7), ("w1", 10), ("w1", 11),
]
_RELU_ACT = {0, 3, 5, 8, 10}  # 5 relu pairs on ACT, 7 on DVE


def _kernel_body_v7(tc, out, xt, pt, w1p, w2t, bpn, b2c, corr, meanc):
    from concourse import mybir

    nc = tc.nc
    f32 = mybir.dt.float32
    bf16 = mybir.dt.bfloat16
    fp8 = mybir.dt.float8e4
    AF = mybir.ActivationFunctionType
    ALU = mybir.AluOpType
    DR = mybir.MatmulPerfMode.DoubleRow
    DP = mybir.MatmulPerfMode.DoublePixel if USE_DP else None

    with (
        tc.tile_pool(name="weights", bufs=1) as wpool,
        tc.tile_pool(name="xload", bufs=2) as xpool,
        tc.tile_pool(name="hicat", bufs=2) as hicpool,
        tc.tile_pool(name="usb", bufs=2) as upool,
        tc.tile_pool(name="empool", bufs=2) as empool,
        tc.tile_pool(name="scr", bufs=2) as scrpool,
        tc.tile_pool(name="small", bufs=2) as small,
        tc.tile_pool(name="vout", bufs=3) as vpool,
        tc.tile_pool(name="psum_hi", bufs=1, space="PSUM") as psum_hi,
        tc.tile_pool(name="psum_u", bufs=2, space="PSUM") as psum_u,
        tc.tile_pool(name="psum_a", bufs=2, space="PSUM") as psum_a,
    ):
        # ---- weights / constants: host-packed partition-major blobs, one
        # DMA each, on the scalar queue so sync is free for batch-0 work ----
        pt_sb = wpool.tile([128, DC, NH * HD], fp8)  # 4*P^T [d_in, d_chunk, g]
        nc.scalar.dma_start(out=pt_sb, in_=pt)
        # 64*W1^T per head with 64*b1' as a 97th contraction row
        w1_sb = wpool.tile([HD + 1, NH, HID], fp8)
        nc.gpsimd.dma_start(out=w1_sb, in_=w1p)
        w2_sb = wpool.tile([128, NH, FC, HD], fp8)  # 64*W2 [f_in, head, fc, h]
        nc.gpsimd.dma_start(out=w2_sb, in_=w2t)
        bpn_sb = wpool.tile([HD, NH], f32)  # per-head bP, re-added at the end
        nc.scalar.dma_start(out=bpn_sb, in_=bpn)
        b2_sb = wpool.tile([HD, NH, 1], f32)
        nc.gpsimd.dma_start(out=b2_sb, in_=b2c)
        corr_sb = wpool.tile([HD, BPC, NH], f32)
        nc.scalar.dma_start(out=corr_sb, in_=corr)
        mean_sb = wpool.tile([HD, BPC, NH], f32)  # exact host sum_real hi
        nc.gpsimd.dma_start(out=mean_sb, in_=meanc)

        # hi, manually triple-buffered (3-deep pipeline): [97 partitions, buf,
        # head, seq], holding 4*hi in fp8. Row 96 is a constant 4.0 (the b1'
        # row at matching scale), written once.
        hi_all = wpool.tile([HD + 1, 3, NH, S], fp8)
        nc.gpsimd.memset(hi_all[HD:HD + 1, :, :, :], 4.0)

        out_r = out.rearrange("b (nh hd) -> b hd nh", nh=NH)
        xt_r = xt.rearrange("b (c p) s -> b p c s", p=128)

        state = {}

        def issue_x_dma(b):
            xt_sb = xpool.tile([128, DC, S], fp8, tag="xt", name=f"xt_{b}")
            nc.sync.dma_start(out=xt_sb, in_=xt_r[b])
            state[("x", b)] = xt_sb

        def proj_pair(b, tp, st):
            hi_ps = psum_hi.tile([128, 2, S], f32, tag="hi")
            for half in range(2):
                t = tp * 2 + half
                for jp in range(DC // 2):
                    nc.tensor.matmul(
                        hi_ps[:, half, :],
                        lhsT=pt_sb[:, 2 * jp:2 * jp + 2, t * 128:(t + 1) * 128],
                        rhs=state[("x", b)][:, 2 * jp:2 * jp + 2, :],
                        start=(jp == 0),
                        stop=(jp == DC // 2 - 1),
                        perf_mode=DR,
                    )
            # paired evac on ACT: hi8 = fp8(4*hi) (P was host-scaled by 4)
            nc.scalar.activation(
                out=st["hic"][:, tp * 2:tp * 2 + 2, :], in_=hi_ps,
                func=AF.Copy, scale=1.0,
            )
            # repartition the pieces this evac produced (sync queue)
            for (i, t, base, ln, off) in _PIECES_BY_TP[tp]:
                nc.sync.dma_start(
                    out=hi_all[off:off + ln, b % 3, i, :],
                    in_=st["hic"][base:base + ln, t, :],
                )

        def w2_head(o, st, i):
            a_ps = psum_a.tile([HD, S], f32, tag="a")
            nc.tensor.matmul(
                a_ps,
                lhsT=w2_sb[:, i, 0:2, :],
                rhs=st["u"][:, i * FC:i * FC + 2, :],
                start=True,
                stop=False,
                perf_mode=DR,
            )
            nc.tensor.matmul(
                a_ps,
                lhsT=w2_sb[:, i, 2, :],
                rhs=st["u"][:, i * FC + 2, :],
                start=False,
                stop=True,
                perf_mode=DP,
            )
            nc.scalar.activation(
                out=st["em"][:, i, :], in_=a_ps, func=AF.Exp,
                bias=b2_sb[:, i, :], scale=0.0009765625,
                accum_out=st["den"][:, i:i + 1],
            )
            # correction term in ONE DVE op: scr = (em - 1) * hi8, accum
            scr = scrpool.tile([HD, S], bf16, tag="scr")
            nc.vector.scalar_tensor_tensor(
                out=scr, in0=st["em"][:, i, :], scalar=-1.0,
                in1=hi_all[:HD, o % 3, i, :],
                op0=ALU.add, op1=ALU.mult,
                accum_out=st["vnum"][:, i:i + 1],
            )

        def w1_pair(m, st, cp):
            u_ps = psum_u.tile([128, 2, S], f32, tag="u")
            for half in range(2):
                ch = cp * 2 + half
                i, fc = divmod(ch, FC)
                nc.tensor.matmul(
                    u_ps[:, half, :],
                    lhsT=w1_sb[:, i, fc * 128:(fc + 1) * 128],
                    rhs=hi_all[:, m % 3, i, :],
                    start=True,
                    stop=True,
                    perf_mode=DP,
                )  # K=97 (the 97th row adds 64*b1')
            # paired relu evac, u8 = fp8(16u) = fp8(relu(psum) / 16)
            # (psum holds 256*u_pre: 64x from W1, 4x from hi)
            if cp in _RELU_ACT:
                nc.scalar.activation(
                    out=st["u"][:, cp * 2:cp * 2 + 2, :], in_=u_ps,
                    func=AF.Relu, scale=0.0625,
                )
            else:
                nc.vector.tensor_scalar(
                    out=st["u"][:, cp * 2:cp * 2 + 2, :], in0=u_ps,
                    scalar1=0.0, scalar2=0.0625,
                    op0=ALU.max, op1=ALU.mult,
                )

        def tail(o, st):
            # vnum and mean carry a 4x scale (hi8 = 4*hi, meanc = 4*mean);
            # the final STT divides it back out: vout = vq4 * 0.25 + bP.
            den2 = small.tile([HD, NH], f32, tag="den2")
            nc.gpsimd.tensor_sub(den2, st["den"], corr_sb[:, o, :])
            rden = small.tile([HD, NH], f32, tag="rden")
            nc.vector.reciprocal(rden, den2)
            vn2 = small.tile([HD, NH], f32, tag="vn2")
            nc.gpsimd.tensor_add(vn2, st["vnum"], mean_sb[:, o, :])
            vq4 = small.tile([HD, NH], f32, tag="vq4")
            nc.gpsimd.tensor_mul(vq4, vn2, rden)
            vout = vpool.tile([HD, NH], f32, tag="vout")
            nc.vector.scalar_tensor_tensor(
                out=vout, in0=vq4, scalar=0.25, in1=bpn_sb,
                op0=ALU.mult, op1=ALU.add,
            )
            nc.sync.dma_start(out=out_r[o], in_=vout)

        issue_x_dma(0)
        for it in range(BPC + 2):
            b = it if it < BPC else None                  # proj batch
            m = it - 1 if 0 <= it - 1 < BPC else None     # W1 batch
            o = it - 2 if it >= 2 else None               # score/output batch
            if b is not None and b + 1 < BPC:
                issue_x_dma(b + 1)
            if b is not None:
                state.setdefault(b, {})["hic"] = hicpool.tile(
                    [128, HT, S], fp8, tag="hic", name=f"hic_{b}")
            if m is not None:
                state[m]["u"] = upool.tile([128, NCH, S], fp8, tag="u",
                                           name=f"u_sb_{m}")
            if o is not None:
                sto = state[o]
                sto["em"] = empool.tile([HD, NH, S], f32, tag="em",
                                        name=f"em_{o}")
                sto["den"] = small.tile([HD, NH], f32, tag="den",
                                        name=f"den_{o}")
                sto["vnum"] = small.tile([HD, NH], f32, tag="vnum",
                                         name=f"vnum_{o}")
            for kind, idx in _PE_ORDER:
                if kind == "h" and o is not None:
                    w2_head(o, sto, idx)
                elif kind == "w1" and m is not None:
                    w1_pair(m, state[m], idx)
                elif kind == "pair" and b is not None:
                    proj_pair(b, idx, state[b])
            if o is not None:
                tail(o, sto)
                state.pop(o)
                state.pop(("x", o), None)


def build_module(enable_asserts=False):
    """Build + compile the per-core Bass module (same program all 8 cores)."""
    import concourse.bacc as bacc
    import concourse.tile as tile
    from concourse import mybir

    f32 = mybir.dt.float32
    fp8 = mybir.dt.float8e4

    nc = bacc.Bacc(
        "TRN2",
        target_bir_lowering=False,
        debug=False,
        enable_asserts=enable_asserts,
        num_devices=NCORES,
    )
    xt = nc.dram_tensor("xt", [BPC, D, S], fp8, kind="ExternalInput").ap()
    pt = nc.dram_tensor("pt", [128, DC, NH * HD], fp8, kind="ExternalInput").ap()
    w1p = nc.dram_tensor("w1p", [HD + 1, NH, HID], fp8, kind="ExternalInput").ap()
    w2t = nc.dram_tensor("w2t", [128, NH, FC, HD], fp8, kind="ExternalInput").ap()
    bpn = nc.dram_tensor("bpn", [HD, NH], f32, kind="ExternalInput").ap()
    b2c = nc.dram_tensor("b2c", [HD, NH, 1], f32, kind="ExternalInput").ap()
    corr = nc.dram_tensor("corr", [HD, BPC, NH], f32, kind="ExternalInput").ap()
    meanc = nc.dram_tensor("meanc", [HD, BPC, NH], f32, kind="ExternalInput").ap()
    out = nc.dram_tensor("out", [BPC, NH * HD], f32, kind="ExternalOutput").ap()

    with tile.TileContext(nc) as tc:
        _kernel_body_v7(tc, out, xt, pt, w1p, w2t, bpn, b2c, corr, meanc)
    nc.compile()
    return nc


def prep_inputs(token_embeddings, attention_mask, P, bP, W1, b1, W2, b2):
    """Host-side layout prep -> list of 8 per-core input maps."""
    f8 = ml_dtypes.float8_e4m3
    te = np.asarray(token_embeddings, np.float32)
    am = np.asarray(attention_mask, np.float32)
    P_ = np.asarray(P, np.float32)
    bP_ = np.asarray(bP, np.float32)
    W1_ = np.asarray(W1, np.float32)
    b1_ = np.asarray(b1, np.float32)
    W2_ = np.asarray(W2, np.float32)
    b2_ = np.asarray(b2, np.float32)

    # X^T, masked (padded columns zeroed), fp8
    xm_f = te * am[:, :, None]                      # [B, S, D] fp32
    xm = np.ascontiguousarray(xm_f.transpose(0, 2, 1)).astype(f8)  # [B, D, S]

    # pt packed [128, DC, G]: pt[p, dc, g] = 4*P^T[dc*128+p, g], fp8
    # (x4, not x64: the proj PSUM is repartitioned to fp8 hi_all by casting
    # DMA with no rescale, so hi8 = fp8(4*hi) must itself be in range)
    ptT = P_.reshape(NH * HD, D).T * 4.0  # [D, G]
    pt = np.ascontiguousarray(
        np.clip(ptT.reshape(DC, 128, NH * HD).transpose(1, 0, 2), -240.0, 240.0)
    ).astype(f8)

    # w2 packed [128, NH, FC, HD], scaled x64, trn-fp8e4 (clip +-240)
    w2t_ = W2_.transpose(0, 2, 1) * 64.0  # [NH, HID, HD]
    w2t = np.ascontiguousarray(
        np.clip(w2t_.reshape(NH, FC, 128, HD).transpose(2, 0, 1, 3),
                -240.0, 240.0)
    ).astype(f8)
    b2c = np.ascontiguousarray(b2_.reshape(NH, HD).T[:, :, None])  # [HD, NH, 1]
    bpn = np.ascontiguousarray(bP_.reshape(NH, HD).T)  # [HD, NH]

    # b1' = b1 + W1 @ bP (softmax weights sum to 1, so bP moves to the end);
    # w1 packed [97, NH, HID] with 64*b1' as the 97th contraction row, fp8
    b1p = b1_ + np.einsum('ihf,ih->if', W1_.transpose(0, 2, 1), bP_)  # [NH, HID]
    w1p = np.zeros((HD + 1, NH, HID), np.float32)
    w1p[:HD] = W1_.transpose(0, 2, 1).transpose(1, 0, 2) * 64.0
    w1p[HD] = b1p * 64.0
    w1p = np.clip(w1p, -240.0, 240.0).astype(f8)

    # ---- padded-column denominator correction (host, replicating the fp8
    # pipeline): hi8 at a padded column is exactly 0 (X was masked), so
    # psum_u = 64*b1'_q, u_pad = fp8(relu(psum)/4), score = W2q@u/1024 + b2.
    b1q64 = w1p[HD].astype(np.float32)                            # [NH, HID]
    u_pad = (np.maximum(b1q64, 0.0) * 0.25).astype(f8).astype(np.float32)
    w2qf = w2t.astype(np.float32)                                 # [128,NH,FC,HD]
    a_pad = (np.einsum('pifh,ifp->ih', w2qf, u_pad.reshape(NH, FC, 128))
             / 1024.0 + b2_.reshape(NH, HD))
    em_pad = np.exp(a_pad)                                        # [NH, HD]
    n_pad = (S - am.sum(axis=1)).astype(np.float32)               # [B]
    corr_f = n_pad[:, None, None] * em_pad[None]                  # [B, NH, HD]

    # ---- exact mean term: sum over real tokens of hi0 = P @ sum_s X,
    # shipped x4 to match the 4x scale of the on-chip vnum accumulator
    xsum = xm_f.sum(axis=1, dtype=np.float64)                     # [B, D]
    mean_f = np.einsum('ihd,bd->bih', P_.astype(np.float64) * 4.0, xsum)

    in_maps = []
    for c in range(NCORES):
        sl = slice(c * BPC, (c + 1) * BPC)
        in_maps.append(
            {
                "xt": np.ascontiguousarray(xm[sl]),
                "pt": pt,
                "w1p": w1p,
                "w2t": w2t,
                "bpn": bpn,
                "b2c": b2c,
                "corr": np.ascontiguousarray(
                    corr_f[sl].transpose(2, 0, 1).astype(np.float32)
                ),
                "meanc": np.ascontiguousarray(
                    mean_f[sl].transpose(2, 0, 1).astype(np.float32)
                ),
            }
        )
    return in_maps


def kernel(**inputs):
    if "nc" not in _CACHE:
        _CACHE["nc"] = build_module()
    nc = _CACHE["nc"]
    in_maps = prep_inputs(**inputs)
    from concourse.bass_utils import run_bass_kernel_spmd

    res = run_bass_kernel_spmd(nc, in_maps, core_ids=list(range(NCORES)))
    outs = [np.asarray(res.results[c]["out"], np.float32) for c in range(NCORES)]
    return np.concatenate(outs, axis=0)


# revision 35
# speedup vs baseline: 1.0310x; 1.0310x over previous
"""Trainium2 Bass kernel for MultiHeadGeneralizedPooling.

Reference computation (per batch b):
  Hi   = einsum('sd,ihd->ish', X, P) + bP             (nh, S, HD)
  A    = W2 @ relu(W1 @ Hi + b1) + b2                 (nh, S, HD)
  A    = softmax(A + log(mask), axis=S)
  v    = sum_s Hi * A                                 (nh, HD)
  out  = concat_heads(v)                              (NH*HD,)

v7 strategy (evolved from the v6 bf16 baseline, kernel_v6_baseline.py;
337-375us -> ~280us):
  - KEY ALGEBRA: with em = exp(score), v_num = sum_s em*hi
      = sum_real hi  +  sum_s (em-1)*hi.
    The first term is computed EXACTLY on the host (fp32 P @ sum_s X).
    The second term carries an (em-1) ~ O(0.03) weight, so fp8 noise in
    hi contributes only ~4e-5 to v. This makes the ENTIRE on-chip
    pipeline fp8-tolerant: X, P, hi, W1, W2 all fp8.
  - fp8 DoubleRow projection: K=768 contraction packed as 3 matmuls of
    K=256 (2 k-tiles/partition) per output tile -> 18 matmuls/batch at
    0.5 cycles/moving-elem, HALF the bf16 PE time. (DoublePixel was
    measured to be a silent no-op on TRN2; fp8 without a perf mode gains
    nothing.)
  - X shipped as fp8 (half the HBM traffic of v6), one DMA per batch on
    the sync queue, prefetched one iteration ahead.
  - hi stored fp8-only as 4*hi (P host-scaled x4): paired ACT evac from
    PSUM, fp8 repartition pieces on sync (half the bytes of v6).
  - scores: W1 fp8x64 (64*b1' as 97th contraction row against the
    constant-4.0 hi row), relu evac scale 1/16 -> u = fp8(16u), W2
    fp8x64 DoubleRow, exp scale 1/1024 on ACT with f32 em + den accum.
  - weighted sum: one DVE STT per head: (em - 1) * hi8, free-dim
    accumulated -> vnum4.  v = (vnum4 + 4*mean_host)/(4*(den - corr))*4
    folded as vout = (vq4 * 0.25) + bP in a single STT.
  - 3-deep software pipeline: iteration it runs proj(b=it) | W1(b-1) |
    W2+softmax+output(b-2), fully interleaved on PE (_PE_ORDER) so every
    PSUM WAR gap is covered by ready work from another stream. Engine
    split: ACT = hi evacs(3 pairs) + exp(8) + 5 relu pairs; DVE = 7 relu
    pairs + STT(8) + reciprocal + final STT; Pool = tail elementwise;
    sync = X + repartition(15) + out. ACT/DVE are the ~15us/batch
    co-bottleneck (Pool cannot read PSUM on TRN2).
  - softmax without max-subtraction (scores ~N(0,0.03)); padded-column
    denominator overcount subtracted via host-computed corr (replicates
    the chip's exact fp8 arithmetic on a padded column).
"""

import numpy as np
import ml_dtypes

B, S, D = 128, 512, 768
NH, HD = 8, 96
HID = 4 * HD  # 384
NCORES = 8
BPC = B // NCORES  # batches per core
DC = D // 128      # 6 d-chunks
FC = HID // 128    # 3 f-chunks
HT = D // 128      # 6 concat feature tiles
NCH = NH * FC      # 24 u-chunks per batch

# fp8 DoublePixel perf mode (2 moving pixels/cycle) on the K<=128 matmuls
# (W1, W2 third chunk). CoreSim doesn't model DP, so test.py sim sets this
# False before build_module; hardware correctness is gated by rel-err.
USE_DP = False  # measured: no effect on TRN2 (silently ignored), keep off

_CACHE = {}


def _lattice_split(base, length):
    segs = []
    while length > 0:
        for sz in (128, 96, 64, 32):
            if length >= sz and (base == 0 if sz == 96 else base % sz == 0):
                segs.append((base, sz))
                base += sz
                length -= sz
                break
        else:
            raise ValueError((base, length))
    return segs


# head i occupies concatenated-feature rows [96i, 96i+96): pieces of the six
# 128-row tiles: (tile, base_partition, length, head_row_offset)
_PIECES = []
for _i in range(NH):
    lo, hi = _i * HD, (_i + 1) * HD
    ps = []
    t0, t1 = lo // 128, (hi - 1) // 128
    for _t in range(t0, t1 + 1):
        s = max(lo, _t * 128)
        e = min(hi, (_t + 1) * 128)
        for _b, _sz in _lattice_split(s - _t * 128, e - s):
            ps.append((_t, _b, _sz, _t * 128 + _b - lo))
    _PIECES.append(ps)

# pieces grouped by the tile-pair whose evac produces them
_PIECES_BY_TP = [[], [], []]
for _i in range(NH):
    for (_t, _b, _sz, _off) in _PIECES[_i]:
        _PIECES_BY_TP[_t // 2].append((_i, _t, _b, _sz, _off))

# PE issue order per iteration of the 3-deep pipeline: W2 heads (batch o),
# W1 chunk-pairs (batch m) and proj tile-pairs (batch b) interleaved so every
# PSUM WAR gap (psum_hi evac, psum_u relu, psum_a exp) is covered by ready
# work from another stream.
_PE_ORDER = [
    ("h", 0), ("w1", 0), ("pair", 0), ("h", 1), ("w1", 1), ("h", 2),
    ("w1", 2), ("pair", 1), ("h", 3), ("w1", 3), ("h", 4), ("w1", 4),
    ("pair", 2), ("h", 5), ("w1", 5), ("h", 6), ("w1", 6), ("h", 7),
    ("w1", 7), ("w1", 8), ("w1", 9), ("w1", 10), ("w1", 11),
]
_RELU_ACT = {0, 3, 5, 8, 10}  # 5 relu pairs on ACT, 7 on DVE


def _kernel_body_v7(tc, out, xt, pt, w1p, w2t, bpn, b2c, corr, meanc):
    from concourse import mybir

    nc = tc.nc
    f32 = mybir.dt.float32
    bf16 = mybir.dt.bfloat16
    fp8 = mybir.dt.float8e4
    AF = mybir.ActivationFunctionType
    ALU = mybir.AluOpType
    DR = mybir.MatmulPerfMode.DoubleRow
    DP = mybir.MatmulPerfMode.DoublePixel if USE_DP else None

    with (
        tc.tile_pool(name="weights", bufs=1) as wpool,
        tc.tile_pool(name="xload", bufs=2) as xpool,
        tc.tile_pool(name="hicat", bufs=2) as hicpool,
        tc.tile_pool(name="usb", bufs=2) as upool,
        tc.tile_pool(name="empool", bufs=2) as empool,
        tc.tile_pool(name="scr", bufs=2) as scrpool,
        tc.tile_pool(name="small", bufs=2) as small,
        tc.tile_pool(name="vout", bufs=3) as vpool,
        tc.tile_pool(name="psum_hi", bufs=1, space="PSUM") as psum_hi,
        tc.tile_pool(name="psum_u", bufs=2, space="PSUM") as psum_u,
        tc.tile_pool(name="psum_a", bufs=2, space="PSUM") as psum_a,
    ):
        # ---- weights / constants: host-packed partition-major blobs, one
        # DMA each, on the scalar queue so sync is free for batch-0 work ----
        pt_sb = wpool.tile([128, DC, NH * HD], fp8)  # 4*P^T [d_in, d_chunk, g]
        nc.scalar.dma_start(out=pt_sb, in_=pt)
        # 64*W1^T per head with 64*b1' as a 97th contraction row
        w1_sb = wpool.tile([HD + 1, NH, HID], fp8)
        nc.gpsimd.dma_start(out=w1_sb, in_=w1p)
        w2_sb = wpool.tile([128, NH, FC, HD], fp8)  # 64*W2 [f_in, head, fc, h]
        nc.gpsimd.dma_start(out=w2_sb, in_=w2t)
        bpn_sb = wpool.tile([HD, NH], f32)  # per-head bP, re-added at the end
        nc.scalar.dma_start(out=bpn_sb, in_=bpn)
        b2_sb = wpool.tile([HD, NH, 1], f32)
        nc.gpsimd.dma_start(out=b2_sb, in_=b2c)
        corr_sb = wpool.tile([HD, BPC, NH], f32)
        nc.scalar.dma_start(out=corr_sb, in_=corr)
        mean_sb = wpool.tile([HD, BPC, NH], f32)  # exact host sum_real hi
        nc.gpsimd.dma_start(out=mean_sb, in_=meanc)

        # hi, manually triple-buffered (3-deep pipeline): [97 partitions, buf,
        # head, seq], holding 4*hi in fp8. Row 96 is a constant 4.0 (the b1'
        # row at matching scale), written once.
        hi_all = wpool.tile([HD + 1, 3, NH, S], fp8)
        nc.gpsimd.memset(hi_all[HD:HD + 1, :, :, :], 4.0)

        out_r = out.rearrange("b (nh hd) -> b hd nh", nh=NH)
        xt_r = xt.rearrange("b (c p) s -> b p c s", p=128)

        state = {}

        def issue_x_dma(b):
            xt_sb = xpool.tile([128, DC, S], fp8, tag="xt", name=f"xt_{b}")
            nc.sync.dma_start(out=xt_sb, in_=xt_r[b])
            state[("x", b)] = xt_sb

        def proj_pair(b, tp, st):
            hi_ps = psum_hi.tile([128, 2, S], f32, tag="hi")
            for half in range(2):
                t = tp * 2 + half
                for jp in range(DC // 2):
                    nc.tensor.matmul(
                        hi_ps[:, half, :],
                        lhsT=pt_sb[:, 2 * jp:2 * jp + 2, t * 128:(t + 1) * 128],
                        rhs=state[("x", b)][:, 2 * jp:2 * jp + 2, :],
                        start=(jp == 0),
                        stop=(jp == DC // 2 - 1),
                        perf_mode=DR,
                    )
            # paired evac on ACT: hi8 = fp8(4*hi) (P was host-scaled by 4)
            nc.scalar.activation(
                out=st["hic"][:, tp * 2:tp * 2 + 2, :], in_=hi_ps,
                func=AF.Copy, scale=1.0,
            )
            # repartition the pieces this evac produced (sync queue)
            for (i, t, base, ln, off) in _PIECES_BY_TP[tp]:
                nc.sync.dma_start(
                    out=hi_all[off:off + ln, b % 3, i, :],
                    in_=st["hic"][base:base + ln, t, :],
                )

        def w2_head(o, st, i):
            a_ps = psum_a.tile([HD, S], f32, tag="a")
            nc.tensor.matmul(
                a_ps,
                lhsT=w2_sb[:, i, 0:2, :],
                rhs=st["u"][:, i * FC:i * FC + 2, :],
                start=True,
                stop=False,
                perf_mode=DR,
            )
            nc.tensor.matmul(
                a_ps,
                lhsT=w2_sb[:, i, 2, :],
                rhs=st["u"][:, i * FC + 2, :],
                start=False,
                stop=True,
                perf_mode=DP,
            )
            nc.scalar.activation(
                out=st["em"][:, i, :], in_=a_ps, func=AF.Exp,
                bias=b2_sb[:, i, :], scale=0.0009765625,
                accum_out=st["den"][:, i:i + 1],
            )
            # correction term in ONE DVE op: scr = (em - 1) * hi8, accum
            scr = scrpool.tile([HD, S], bf16, tag="scr")
            nc.vector.scalar_tensor_tensor(
                out=scr, in0=st["em"][:, i, :], scalar=-1.0,
                in1=hi_all[:HD, o % 3, i, :],
                op0=ALU.add, op1=ALU.mult,
                accum_out=st["vnum"][:, i:i + 1],
            )

        def w1_pair(m, st, cp):
            u_ps = psum_u.tile([128, 2, S], f32, tag="u")
            for half in range(2):
                ch = cp * 2 + half
                i, fc = divmod(ch, FC)
                nc.tensor.matmul(
                    u_ps[:, half, :],
                    lhsT=w1_sb[:, i, fc * 128:(fc + 1) * 128],
                    rhs=hi_all[:, m % 3, i, :],
                    start=True,
                    stop=True,
                    perf_mode=DP,
                )  # K=97 (the 97th row adds 64*b1')
            # paired relu evac, u8 = fp8(16u) = fp8(relu(psum) / 16)
            # (psum holds 256*u_pre: 64x from W1, 4x from hi)
            if cp in _RELU_ACT:
                nc.scalar.activation(
                    out=st["u"][:, cp * 2:cp * 2 + 2, :], in_=u_ps,
                    func=AF.Relu, scale=0.0625,
                )
            else:
                nc.vector.tensor_scalar(
                    out=st["u"][:, cp * 2:cp * 2 + 2, :], in0=u_ps,
                    scalar1=0.0, scalar2=0.0625,
                    op0=ALU.max, op1=ALU.mult,
                )

        def tail(o, st):
            # vnum and mean carry a 4x scale (hi8 = 4*hi, meanc = 4*mean);
            # the final STT divides it back out: vout = vq4 * 0.25 + bP.
            den2 = small.tile([HD, NH], f32, tag="den2")
            nc.gpsimd.tensor_sub(den2, st["den"], corr_sb[:, o, :])
            rden = small.tile([HD, NH], f32, tag="rden")
            nc.vector.reciprocal(rden, den2)
            vn2 = small.tile([HD, NH], f32, tag="vn2")
            nc.gpsimd.tensor_add(vn2, st["vnum"], mean_sb[:, o, :])
            vq4 = small.tile([HD, NH], f32, tag="vq4")
            nc.gpsimd.tensor_mul(vq4, vn2, rden)
            vout = vpool.tile([HD, NH], f32, tag="vout")
            nc.vector.scalar_tensor_tensor(
                out=vout, in0=vq4, scalar=0.25, in1=bpn_sb,
                op0=ALU.mult, op1=ALU.add,
            )
            nc.sync.dma_start(out=out_r[o], in_=vout)

        issue_x_dma(0)
        for it in range(BPC + 2):
            b = it if it < BPC else None                  # proj batch
            m = it - 1 if 0 <= it - 1 < BPC else None     # W1 batch
            o = it - 2 if it >= 2 else None               # score/output batch
            if b is not None and b + 1 < BPC:
                issue_x_dma(b + 1)
            if b is not None:
                state.setdefault(b, {})["hic"] = hicpool.tile(
                    [128, HT, S], fp8, tag="hic", name=f"hic_{b}")
            if m is not None:
                state[m]["u"] = upool.tile([128, NCH, S], fp8, tag="u",
                                           name=f"u_sb_{m}")
            if o is not None:
                sto = state[o]
                sto["em"] = empool.tile([HD, NH, S], f32, tag="em",
                                        name=f"em_{o}")
                sto["den"] = small.tile([HD, NH], f32, tag="den",
                                        name=f"den_{o}")
                sto["vnum"] = small.tile([HD, NH], f32, tag="vnum",
                                         name=f"vnum_{o}")
            for kind, idx in _PE_ORDER:
                if kind == "h" and o is not None:
                    w2_head(o, sto, idx)
                elif kind == "w1" and m is not None:
                    w1_pair(m, state[m], idx)
                elif kind == "pair" and b is not None:
                    proj_pair(b, idx, state[b])
            if o is not None:
                tail(o, sto)
                state.pop(o)
                state.pop(("x", o), None)


def build_module(enable_asserts=False):
    """Build + compile the per-core Bass module (same program all 8 cores)."""
    import concourse.bacc as bacc
    import concourse.tile as tile
    from concourse import mybir

    f32 = mybir.dt.float32
    fp8 = mybir.dt.float8e4

    nc = bacc.Bacc(
        "TRN2",
        target_bir_lowering=False,
        debug=False,
        enable_asserts=enable_asserts,
        num_devices=NCORES,
    )
    xt = nc.dram_tensor("xt", [BPC, D, S], fp8, kind="ExternalInput").ap()
    pt = nc.dram_tensor("pt", [128, DC, NH * HD], fp8, kind="ExternalInput").ap()
    w1p = nc.dram_tensor("w1p", [HD + 1, NH, HID], fp8, kind="ExternalInput").ap()
    w2t = nc.dram_tensor("w2t", [128, NH, FC, HD], fp8, kind="ExternalInput").ap()
    bpn = nc.dram_tensor("bpn", [HD, NH], f32, kind="ExternalInput").ap()
    b2c = nc.dram_tensor("b2c", [HD, NH, 1], f32, kind="ExternalInput").ap()
    corr = nc.dram_tensor("corr", [HD, BPC, NH], f32, kind="ExternalInput").ap()
    meanc = nc.dram_tensor("meanc", [HD, BPC, NH], f32, kind="ExternalInput").ap()
    out = nc.dram_tensor("out", [BPC, NH * HD], f32, kind="ExternalOutput").ap()

    with tile.TileContext(nc) as tc:
        _kernel_body_v7(tc, out, xt, pt, w1p, w2t, bpn, b2c, corr, meanc)
    nc.compile()
    return nc


def prep_inputs(token_embeddings, attention_mask, P, bP, W1, b1, W2, b2):
    """Host-side layout prep -> list of 8 per-core input maps."""
    f8 = ml_dtypes.float8_e4m3
    te = np.asarray(token_embeddings, np.float32)
    am = np.asarray(attention_mask, np.float32)
    P_ = np.asarray(P, np.float32)
    bP_ = np.asarray(bP, np.float32)
    W1_ = np.asarray(W1, np.float32)
    b1_ = np.asarray(b1, np.float32)
    W2_ = np.asarray(W2, np.float32)
    b2_ = np.asarray(b2, np.float32)

    # X^T, masked (padded columns zeroed), fp8
    xm_f = te * am[:, :, None]                      # [B, S, D] fp32
    xm = np.ascontiguousarray(xm_f.transpose(0, 2, 1)).astype(f8)  # [B, D, S]

    # pt packed [128, DC, G]: pt[p, dc, g] = 4*P^T[dc*128+p, g], fp8
    # (x4, not x64: the proj PSUM is repartitioned to fp8 hi_all by casting
    # DMA with no rescale, so hi8 = fp8(4*hi) must itself be in range)
    ptT = P_.reshape(NH * HD, D).T * 4.0  # [D, G]
    pt = np.ascontiguousarray(
        np.clip(ptT.reshape(DC, 128, NH * HD).transpose(1, 0, 2), -240.0, 240.0)
    ).astype(f8)

    # w2 packed [128, NH, FC, HD], scaled x64, trn-fp8e4 (clip +-240)
    w2t_ = W2_.transpose(0, 2, 1) * 64.0  # [NH, HID, HD]
    w2t = np.ascontiguousarray(
        np.clip(w2t_.reshape(NH, FC, 128, HD).transpose(2, 0, 1, 3),
                -240.0, 240.0)
    ).astype(f8)
    b2c = np.ascontiguousarray(b2_.reshape(NH, HD).T[:, :, None])  # [HD, NH, 1]
    bpn = np.ascontiguousarray(bP_.reshape(NH, HD).T)  # [HD, NH]

    # b1' = b1 + W1 @ bP (softmax weights sum to 1, so bP moves to the end);
    # w1 packed [97, NH, HID] with 64*b1' as the 97th contraction row, fp8
    b1p = b1_ + np.einsum('ihf,ih->if', W1_.transpose(0, 2, 1), bP_)  # [NH, HID]
    w1p = np.zeros((HD + 1, NH, HID), np.float32)
    w1p[:HD] = W1_.transpose(0, 2, 1).transpose(1, 0, 2) * 64.0
    w1p[HD] = b1p * 64.0
    w1p = np.clip(w1p, -240.0, 240.0).astype(f8)

    # ---- padded-column denominator correction (host, replicating the fp8
    # pipeline): hi8 at a padded column is exactly 0 (X was masked), so
    # psum_u = 64*b1'_q, u_pad = fp8(relu(psum)/4), score = W2q@u/1024 + b2.
    b1q64 = w1p[HD].astype(np.float32)                            # [NH, HID]
    u_pad = (np.maximum(b1q64, 0.0) * 0.25).astype(f8).astype(np.float32)
    w2qf = w2t.astype(np.float32)                                 # [128,NH,FC,HD]
    a_pad = (np.einsum('pifh,ifp->ih', w2qf, u_pad.reshape(NH, FC, 128))
             / 1024.0 + b2_.reshape(NH, HD))
    em_pad = np.exp(a_pad)                                        # [NH, HD]
    n_pad = (S - am.sum(axis=1)).astype(np.float32)               # [B]
    corr_f = n_pad[:, None, None] * em_pad[None]                  # [B, NH, HD]

    # ---- exact mean term: sum over real tokens of hi0 = P @ sum_s X,
    # shipped x4 to match the 4x scale of the on-chip vnum accumulator
    xsum = xm_f.sum(axis=1, dtype=np.float64)                     # [B, D]
    mean_f = np.einsum('ihd,bd->bih', P_.astype(np.float64) * 4.0, xsum)

    in_maps = []
    for c in range(NCORES):
        sl = slice(c * BPC, (c + 1) * BPC)
        in_maps.append(
            {
                "xt": np.ascontiguousarray(xm[sl]),
                "pt": pt,
                "w1p": w1p,
                "w2t": w2t,
                "bpn": bpn,
                "b2c": b2c,
                "corr": np.ascontiguousarray(
                    corr_f[sl].transpose(2, 0, 1).astype(np.float32)
                ),
                "meanc": np.ascontiguousarray(
                    mean_f[sl].transpose(2, 0, 1).astype(np.float32)
                ),
            }
        )
    return in_maps


def kernel(**inputs):
    if "nc" not in _CACHE:
        _CACHE["nc"] = build_module()
    nc = _CACHE["nc"]
    in_maps = prep_inputs(**inputs)
    from concourse.bass_utils import run_bass_kernel_spmd

    res = run_bass_kernel_spmd(nc, in_maps, core_ids=list(range(NCORES)))
    outs = [np.asarray(res.results[c]["out"], np.float32) for c in range(NCORES)]
    return np.concatenate(outs, axis=0)
